# revision 1
# baseline (speedup 1.0000x reference)
"""Trainium2 Bass kernel for nn_CondNetCSLoss.

Strategy (pure data parallel, 8 cores):
- Shard batch B=16 -> 2 images per NeuronCore.
- Device kernel (per core): streams pred/target/lab and computes, per
  (image, label) region, the masked reductions
      n      = sum(m)                    S1 = sum(m*pred)
      S2     = sum(m*pred^2)             cx = sum(m[:, :-1]*m[:, 1:])
      Sx     = sum(mx*|dpred_x|)         cy = sum(m[:-1, :]*m[1:, :])
      Sy     = sum(my*|dpred_y|)
  plus sum(|pred-target|), as per-partition partials written to DRAM.
- Host: combines partials, computes CV/smooth terms + validity, computes the
  sampled-pair ranking term (which needs the JAX PRNG stream; the 9600
  input-independent uniforms are embedded below), handles the (practically
  unreachable) erosion fallback exactly, and assembles the scalar loss.
"""

import base64
import zlib

import numpy as np

import concourse.bacc as bacc
import concourse.mybir as mybir
import concourse.tile as tile
from concourse.bass_utils import run_bass_kernel_spmd

# ---- problem constants (hardcoded per spec) --------------------------------
N_CORES = 8
B, H, W = 16, 512, 512
BPC = B // N_CORES          # images per core
NCH = H // 128              # 128-row chunks per image
LABELS = (5, 8, 13)
TGT_CV = (0.077, 0.227, 0.348)
EPS = 1e-6
RANK_PAIRS = 100
W_MEAN = 0.5
W_STD = 0.5

NSTAT = 7                   # n, S1, S2, cx, Sx, cy, Sy
STAT_COLS = BPC * 3 * NSTAT * NCH          # 168
MEAN_COLS = BPC * NCH                      # 8
OUT_COLS = STAT_COLS + MEAN_COLS           # 176

F32 = mybir.dt.float32
BF16 = mybir.dt.bfloat16
I32 = mybir.dt.int32
ALU = mybir.AluOpType
AFT = mybir.ActivationFunctionType

